# revision 28
# baseline (speedup 1.0000x reference)
"""Trainium2 Bass kernel for variable-length prefill GQA attention + KV-cache store.

Problem (nn_Attention_44057774522397):
  q [T=4096, 32, 128] f32, k/v [T, 8, 128] f32, k_cache/v_cache [8192, 8, 128] f32,
  cu_seqlens [5] i32, slot_mapping [T] i32.
  reference: scatter k/v into caches at slot_mapping; causal varlen attention
  (GQA 32q/8kv heads) over the packed sequences.  Returns (out, k_cache, v_cache).

Sharding: 8 cores, head-parallel.  Core c gets q heads [4c:4c+4] and kv head c
(GQA group size 4 -> each core needs exactly one kv head).  slot/cu handled on
host (program structure); caches sharded by kv head.

Device kernel (per core, SPMD):
  - S^T = K_j^T-blocks x Q-stripes via f32r matmuls (N=512 full-speed, ~1.5e-4 rel),
    scores computed transposed [k_rows, q_cols] so no on-device transposes are
    needed anywhere (host pre-transposes q/k into [head_dim, tokens]).
  - P^T = exp(scale * S^T) on ScalarE, batched [128, 3*512] per op, output fp16.
  - causal masking: gpsimd affine_select zeroes invalid (q < k) entries of P^T.
  - PV: out[q,129] += P^T-chunk.T @ [V | ones] in fp16 (FWL weight loads);
    column 128 accumulates the softmax denominator for free.
  - normalize with DVE reciprocal + tensor_scalar_mul, DMA out.
  - cache updates are pure DMAs (slot_mapping is arange in practice).
"""

import math
import numpy as np

NUM_HEADS = 32
NUM_KV_HEADS = 8
HEAD_DIM = 128
SCALE = 1.0 / math.sqrt(128.0)
N_CORES = 8
HPC = NUM_HEADS // N_CORES          # q heads per core
GROUPS = NUM_HEADS // NUM_KV_HEADS  # GQA group size
BLK = 128
WBLKS = 4                           # stripe width in 128-blocks (512 cols)
JB = 2                              # j-blocks per exp batch (2 PSUM banks)

_PROGRAM_CACHE = {}
LAST_RESULTS = None  # BassKernelResults of the most recent device run


def _build_program(seq_lens, T, num_slots, skip_upper):
    import sys
    if "/opt/trn_rl_repo" not in sys.path:
        sys.path.insert(0, "/opt/trn_rl_repo")
    from contextlib import ExitStack
    import concourse.bass as bass
    import concourse.mybir as mybir
    import concourse.tile as tile

    f32 = mybir.dt.float32
    f32r = mybir.dt.float32r
    f16 = mybir.dt.float16

    nc = bass.Bass(trn_type="TRN2")
    qT = nc.dram_tensor("qT", [HPC, HEAD_DIM, T], f16, kind="ExternalInput")
    kT = nc.dram_tensor("kT", [HEAD_DIM, T], f16, kind="ExternalInput")
    vaug = nc.dram_tensor("vaug", [BLK, T // BLK, HEAD_DIM + 1], f16, kind="ExternalInput")
    knat = nc.dram_tensor("knat", [T, HEAD_DIM], f32, kind="ExternalInput")
    vnat = nc.dram_tensor("vnat", [T, HEAD_DIM], f32, kind="ExternalInput")
    kc_in = nc.dram_tensor("kc_in", [num_slots, HEAD_DIM], f32, kind="ExternalInput")
    vc_in = nc.dram_tensor("vc_in", [num_slots, HEAD_DIM], f32, kind="ExternalInput")
    out = nc.dram_tensor("out", [HPC, T, HEAD_DIM], f32, kind="ExternalOutput")
    kc_out = nc.dram_tensor("kc_out", [num_slots, HEAD_DIM], f32, kind="ExternalOutput")
    vc_out = nc.dram_tensor("vc_out", [num_slots, HEAD_DIM], f32, kind="ExternalOutput")

    # packed-sequence offsets (in 128-blocks); all lens are multiples of 128
    seqs = []
    o = 0
    for L in seq_lens:
        if L:
            seqs.append((o, L))
        o += L
    assert o == T

    with ExitStack() as ctx:
        tc = ctx.enter_context(tile.TileContext(nc))

        resident = ctx.enter_context(tc.tile_pool(name="resident", bufs=1))
        kT_r = resident.tile([BLK, T], f16)
        v_sb = resident.tile([BLK, T // BLK, HEAD_DIM + 1], f16)

        # warm the exp table before anything else
        warm = resident.tile([BLK, 1], f32)
        nc.vector.memset(warm, 0.0)
        nc.scalar.activation(out=warm, in_=warm,
                             func=mybir.ActivationFunctionType.Exp, scale=1.0)

        # chunked input loads, in first-use order; q is streamed per stripe
        # (prefetched one stripe ahead inside the main loop)
        IC = T // 4
        nb_all = T // BLK
        prologue_loads = []
        for lo in range(0, T, IC):
            prologue_loads.append((kT_r[:, lo:lo + IC], kT[:, lo:lo + IC]))
            jlo, jhi = lo // BLK, (lo + IC) // BLK
            prologue_loads.append((v_sb[:, jlo:jhi, :], vaug[:, jlo:jhi, :]))
        nc.sync.dma_start(out=prologue_loads[0][0], in_=prologue_loads[0][1])

        # ---- cache copy-through (pure DMA; spread through the main loop
        # so it fills DMA idle slots instead of bursting against the
        # attention input loads on chip-shared HBM) ----
        CH = 512
        cache_dmas = []
        for lo in range(0, T, CH):
            cache_dmas.append((kc_out[lo:lo + CH, :], knat[lo:lo + CH, :]))
            cache_dmas.append((vc_out[lo:lo + CH, :], vnat[lo:lo + CH, :]))
        if not skip_upper:
            for lo in range(T, num_slots, CH):
                cache_dmas.append((kc_out[lo:lo + CH, :], kc_in[lo:lo + CH, :]))
                cache_dmas.append((vc_out[lo:lo + CH, :], vc_in[lo:lo + CH, :]))
        cache_dmas.reverse()  # pop() from the front order

        ppool = ctx.enter_context(tc.tile_pool(name="ppool", bufs=4))
        qpool = ctx.enter_context(tc.tile_pool(name="qpool", bufs=4))
        opool = ctx.enter_context(tc.tile_pool(name="opool", bufs=4))
        spsum = ctx.enter_context(tc.tile_pool(name="spsum", bufs=2, space="PSUM"))
        vpsum = ctx.enter_context(tc.tile_pool(name="vpsum", bufs=4, space="PSUM"))

        # stripe list: (head, seq offset tokens, first i-block, last i-block+1)
        stripes = []
        for h in range(HPC):
            for (o, L) in seqs:
                nb = L // BLK
                for ib0 in range(0, nb, WBLKS):
                    stripes.append((h, o, ib0, min(ib0 + WBLKS, nb)))

        # max P^T tile free size: nj * w is maximized by full stripes: 8 * 512
        MAXPT = 8 * (WBLKS * BLK)

        def emit_batch(stripe, pt, qs, jb):
            """One S^T batch: <=JB k-block matmuls + one exp + diag masks."""
            h, o, ib0, ib1 = stripe
            w = (ib1 - ib0) * BLK
            nj = ib1
            ptv = pt.rearrange("p a b -> p (a b)")
            bsz = min(JB, nj - jb)
            # columns < d_min are never read by any PV group (those (i, j)
            # pairs are anti-causal), so neither compute nor exp them
            d_min = max(0, jb - ib0) * BLK
            stp = spsum.tile([BLK, JB, WBLKS * BLK], f32, tag="stp")
            for jj in range(bsz):
                j = jb + jj
                k0 = o + j * BLK
                nc.tensor.matmul(
                    stp[:, jj, d_min:w],
                    kT_r[:, k0:k0 + BLK],
                    qs[:, d_min:w],
                    start=True, stop=True,
                )
            pt_slice = ptv[:, jb * w:(jb + bsz) * w].rearrange(
                "p (a b) -> p a b", a=bsz)[:, :, d_min:w]
            nc.scalar.activation(
                out=pt_slice,
                in_=stp[:, 0:bsz, d_min:w],
                func=mybir.ActivationFunctionType.Exp,
                scale=float(SCALE),
            )
            # causal mask for any diagonal j in this batch: only the
            # [d, d+128) window is ever read partially masked
            for j in range(max(jb, ib0), jb + bsz):
                d = (j - ib0) * BLK
                pj = ptv[:, j * w + d:j * w + d + BLK]
                nc.gpsimd.affine_select(
                    out=pj, in_=pj,
                    compare_op=mybir.AluOpType.is_ge,
                    fill=0.0, base=0, channel_multiplier=-1,
                    pattern=[[1, BLK]],
                )

        def emit_out_dma(stripe, ob):
            h, o, ib0, ib1 = stripe
            wb = ib1 - ib0
            t0 = o + ib0 * BLK
            nc.sync.dma_start(
                out=out[h, t0:t0 + wb * BLK, :].rearrange(
                    "(a p) d -> p a d", p=BLK),
                in_=ob[:, 0:wb, :],
            )

        # --- matmul-granular software pipeline ------------------------------
        # The exp stream on ScalarE is the bottleneck; the PE FIFO must
        # deliver S^T batches at exp cadence while filling the gaps with PV
        # matmuls.  Pace PV emission at single-matmul granularity so that
        # seq-tail stripes (large triangular PV load) don't burst-starve the
        # exp stream.  Stripe distance <= 3 (pt pool has 4 bufs).
        n_batches_total = sum((s[3] + JB - 1) // JB for s in stripes)
        n_pv_mms_total = sum((i + 1) for s in stripes for i in range(s[2], s[3]))
        ratio = n_pv_mms_total / max(n_batches_total, 1)

        from collections import deque
        bq = deque()          # stripes with A fully emitted, B pending
        cur_b = None          # [stripe, pt, ob, pv, i, j]
        done_b_stripes = [0]
        emitted_a = [0]
        emitted_pv = [0]

        def emit_next_pv_mm():
            nonlocal cur_b
            if cur_b is None:
                if not bq:
                    return False
                stripe, pt = bq.popleft()
                ob = opool.tile([BLK, WBLKS, HEAD_DIM], f32, tag="ob")
                cur_b = [stripe, pt, ob, None, stripe[2], 0]
            stripe, pt, ob, pv, i, j = cur_b
            h, o, ib0, ib1 = stripe
            w = (ib1 - ib0) * BLK
            ptv = pt.rearrange("p a b -> p (a b)")
            ic = (i - ib0) * BLK
            if pv is None:
                pv = vpsum.tile([BLK, HEAD_DIM + 1], f32, tag="pv")
                cur_b[3] = pv
            nc.tensor.matmul(
                pv,
                ptv[:, j * w + ic:j * w + ic + BLK],
                v_sb[:, o // BLK + j, :],
                start=(j == 0), stop=(j == i),
            )
            emitted_pv[0] += 1
            if j < i:
                cur_b[5] = j + 1
                return True
            # group done: normalize into ob
            recip = opool.tile([BLK, 1], f32, tag="recip")
            nc.vector.reciprocal(recip, pv[:, HEAD_DIM:HEAD_DIM + 1])
            nc.vector.tensor_scalar_mul(
                ob[:, i - ib0, :], pv[:, 0:HEAD_DIM], recip)
            if i + 1 < ib1:
                cur_b[3] = None
                cur_b[4] = i + 1
                cur_b[5] = 0
            else:
                emit_out_dma(stripe, ob)
                cur_b = None
                done_b_stripes[0] += 1
            return True

        n_stripes = len(stripes)
        cd_per_stripe = (len(cache_dmas) + n_stripes - 1) // max(n_stripes - 4, 1)

        qs_tiles = {}

        def prefetch_qs(si):
            if si >= n_stripes or si in qs_tiles:
                return
            h, o, ib0, ib1 = stripes[si]
            w = (ib1 - ib0) * BLK
            q0 = o + ib0 * BLK
            qs = qpool.tile([BLK, WBLKS * BLK], f16, tag="qs")
            nc.sync.dma_start(out=qs[:, 0:w], in_=qT[h, :, q0:q0 + w])
            qs_tiles[si] = qs

        prefetch_qs(0)
        # rest of the prologue loads, after the first q stripe
        for dst, srcap in prologue_loads[1:]:
            nc.sync.dma_start(out=dst, in_=srcap)
        prefetch_qs(1)

        for si, stripe in enumerate(stripes):
            h, o, ib0, ib1 = stripe
            pt = ppool.tile([BLK, MAXPT // BLK, BLK], f16, tag="pt")
            prefetch_qs(si + 1)
            prefetch_qs(si + 2)
            qs = qs_tiles.pop(si)
            if si >= 2:
                for _ in range(cd_per_stripe):
                    if cache_dmas:
                        dst, srcap = cache_dmas.pop()
                        nc.gpsimd.dma_start(out=dst, in_=srcap)
            for jb in range(0, ib1, JB):
                emit_batch(stripe, pt, qs, jb)
                emitted_a[0] += 1
                while (emitted_pv[0] + 1) <= ratio * emitted_a[0]:
                    if not emit_next_pv_mm():
                        break
            bq.append((stripe, pt))
            # respect the pt pool depth: at most 3 stripes between the one
            # being A-emitted and the oldest not-fully-consumed one
            while si + 1 - done_b_stripes[0] >= 4:
                if not emit_next_pv_mm():
                    break
        while emit_next_pv_mm():
            pass
        while cache_dmas:
            dst, srcap = cache_dmas.pop()
            nc.gpsimd.dma_start(out=dst, in_=srcap)

    return nc


def _strip_redundant_pe_self_waits(nc, mybir):
    """Remove PE-engine waits on the PE's own completion semaphore.

    Tile emits them when a PSUM slot is rewritten (WAW vs the slot's previous
    tile), but every such slot also carries a wait on the previous tile's
    last *reader* (ACT exp / DVE normalize), and that reader itself waited on
    the old writers -- so the own-sem wait is transitively implied.  Keeping
    it forces the PE sequencer to drain its 64-deep pipeline at every slot
    reuse, serializing the matmul stream."""
    pe = mybir.EngineType.PE
    n = 0
    for func in nc.m.functions:
        for block in func.blocks:
            for inst in block.instructions:
                if inst.engine != pe:
                    continue
                si = inst.sync_info
                if si is None or not si.on_wait:
                    continue
                keep = [w for w in si.on_wait
                        if not (getattr(w, "ant_name", "") or "").startswith("PE_")]
                if len(keep) != len(si.on_wait):
                    inst.sync_info = mybir.SyncInfo(
                        on_wait=keep, on_update=list(si.on_update))
                    n += 1
    return n


def _trim_exit_barrier(nc):
    """Drop the duplicated trailing all-engine barrier after the semaphore
    clear (bass emits the drain+barrier+clear+barrier sequence twice "to be
    safe"); the second EVSEM butterfly costs ~4-6us of pure epilogue."""
    for func in nc.m.functions:
        for block in func.blocks:
            if not block.name.endswith("_end"):
                continue
            idx = None
            for n, inst in enumerate(block.instructions):
                if "EVENT_SEMAPHORE_RANGE_CLEAR" in type(inst).__name__ or                    "RANGE_CLEAR" in str(getattr(inst, "isa_opcode_name", "")) or                    "RANGE_CLEAR" in inst.concise():
                    idx = n
            if idx is not None and idx + 1 < len(block.instructions):
                block.instructions = block.instructions[:idx + 1]
    return


def _legalize_waits(nc, mybir, max_waits=1):
    n_fixed = 0
    for func in nc.m.functions:
        for block in func.blocks:
            new_list = []
            changed = False
            for inst in block.instructions:
                si = inst.sync_info
                if si is not None and si.on_wait and len(si.on_wait) > max_waits:
                    waits = list(si.on_wait)
                    head, tail = waits[:-max_waits], waits[-max_waits:]
                    while head:
                        chunk, head = head[:max_waits], head[max_waits:]
                        nop = mybir.InstNoOp(
                            name=f"waitsplit-{nc.next_id()}", ins=[], outs=[])
                        nop.engine = inst.engine
                        nop.sync_info = mybir.SyncInfo(on_wait=chunk, on_update=[])
                        new_list.append(nop)
                        nc.register_instruction(nop, overwrite=True)
                    inst.sync_info = mybir.SyncInfo(
                        on_wait=tail, on_update=list(si.on_update))
                    changed = True
                    n_fixed += 1
                new_list.append(inst)
            if changed:
                block.instructions = new_list
    return n_fixed


def _get_program(seq_lens, T, num_slots, skip_upper):
    import sys
    if "/opt/trn_rl_repo" not in sys.path:
        sys.path.insert(0, "/opt/trn_rl_repo")
    import concourse.mybir as mybir

    key = (tuple(seq_lens), T, num_slots, skip_upper)
    if key not in _PROGRAM_CACHE:
        nc = _build_program(seq_lens, T, num_slots, skip_upper)
        _strip_redundant_pe_self_waits(nc, mybir)
        _trim_exit_barrier(nc)
        _legalize_waits(nc, mybir, max_waits=1)
        _PROGRAM_CACHE[key] = nc
    return _PROGRAM_CACHE[key]


def _host_reference(q, k, v, k_cache, v_cache, cu_seqlens, slot_mapping):
    """Pure-numpy fallback for input shapes the device program doesn't cover."""
    T = q.shape[0]
    kc = k_cache.copy()
    vc = v_cache.copy()
    valid = slot_mapping >= 0
    kc[slot_mapping[valid]] = k[valid]
    vc[slot_mapping[valid]] = v[valid]
    groups = q.shape[1] // k.shape[1]
    kk = np.repeat(k, groups, axis=1)
    vv = np.repeat(v, groups, axis=1)
    out = np.zeros_like(q)
    for b in range(len(cu_seqlens) - 1):
        s0, s1 = int(cu_seqlens[b]), int(cu_seqlens[b + 1])
        L = s1 - s0
        if L <= 0:
            continue
        qs = q[s0:s1].astype(np.float64)
        ks = kk[s0:s1].astype(np.float64)
        vs = vv[s0:s1].astype(np.float64)
        sc = np.einsum("qhd,khd->hqk", qs, ks) * SCALE
        mask = np.tril(np.ones((L, L), dtype=bool))
        sc = np.where(mask[None], sc, -np.inf)
        sc -= sc.max(axis=-1, keepdims=True)
        p = np.exp(sc)
        p /= p.sum(axis=-1, keepdims=True)
        out[s0:s1] = np.einsum("hqk,khd->qhd", p, vs).astype(q.dtype)
    return out, kc, vc


def _install_ntff_hook():
    """Make `antenv.axon_hooks` importable so run_bass_kernel_spmd(trace=True)
    can capture NTFF profiles under axon.  The image's antenv stub lacks the
    module; recreate it and register the ctypes-based hook from trn_boot.
    Silently degrades (tracing skipped) on any failure."""
    import sys
    import types
    try:
        import antenv  # noqa: F401
        if "antenv.axon_hooks" in sys.modules:
            return
        mod = types.ModuleType("antenv.axon_hooks")
        mod._hook = None

        def set_axon_ntff_profile_hook(h):
            mod._hook = h

        def get_axon_ntff_profile_hook():
            return mod._hook

        mod.set_axon_ntff_profile_hook = set_axon_ntff_profile_hook
        mod.get_axon_ntff_profile_hook = get_axon_ntff_profile_hook
        sys.modules["antenv.axon_hooks"] = mod
        try:
            if "/root/.axon_site" not in sys.path:
                sys.path.append("/root/.axon_site")
            from trn_agent_boot.trn_boot import _ntff_profile_via_ctypes
            mod._hook = _ntff_profile_via_ctypes("/opt/axon/libaxon_pjrt.so")
        except Exception:
            pass
    except Exception:
        pass


def kernel(q, k, v, k_cache, v_cache, cu_seqlens, slot_mapping):
    global LAST_RESULTS
    import os
    import sys
    if "/opt/trn_rl_repo" not in sys.path:
        sys.path.insert(0, "/opt/trn_rl_repo")
    _install_ntff_hook()

    q = np.asarray(q, dtype=np.float32)
    k = np.asarray(k, dtype=np.float32)
    v = np.asarray(v, dtype=np.float32)
    k_cache = np.asarray(k_cache, dtype=np.float32)
    v_cache = np.asarray(v_cache, dtype=np.float32)
    cu = np.asarray(cu_seqlens, dtype=np.int64)
    slots = np.asarray(slot_mapping, dtype=np.int64)

    T, H, D = q.shape
    num_slots = k_cache.shape[0]
    seq_lens = (cu[1:] - cu[:-1]).tolist()

    device_ok = (
        H == NUM_HEADS and D == HEAD_DIM and k.shape[1] == NUM_KV_HEADS
        and T % BLK == 0 and all(L % BLK == 0 and L >= 0 for L in seq_lens)
        and int(cu[0]) == 0 and int(cu[-1]) == T
    )
    slots_arange = bool(np.array_equal(slots, np.arange(T)))
    if not device_ok:
        return _host_reference(q, k, v, k_cache, v_cache, cu_seqlens, slot_mapping)

    from concourse.bass_utils import run_bass_kernel_spmd

    skip_upper = (not np.any(k_cache[T:])) and (not np.any(v_cache[T:]))
    nc = _get_program(tuple(int(L) for L in seq_lens), T, num_slots,
                      bool(skip_upper))

    in_maps = []
    for c in range(N_CORES):
        hs = slice(c * HPC, (c + 1) * HPC)
        qc = q[:, hs, :]                             # [T, 4, 128]
        qT = np.ascontiguousarray(qc.transpose(1, 2, 0)).astype(np.float16)
        kc = k[:, c, :]                              # [T, 128]
        kT = np.ascontiguousarray(kc.T).astype(np.float16)
        vc = v[:, c, :]                              # [T, 128]
        vaug = np.ones((BLK, T // BLK, HEAD_DIM + 1), dtype=np.float16)
        vaug[:, :, :HEAD_DIM] = vc.reshape(T // BLK, BLK, HEAD_DIM).transpose(
            1, 0, 2).astype(np.float16)
        in_maps.append({
            "qT": qT,
            "kT": kT,
            "vaug": vaug,
            "knat": np.ascontiguousarray(kc),
            "vnat": np.ascontiguousarray(vc),
            "kc_in": np.ascontiguousarray(k_cache[:, c, :]),
            "vc_in": np.ascontiguousarray(v_cache[:, c, :]),
        })

    trace = bool(int(os.environ.get("KERNEL_TRACE", "0")))
    res = run_bass_kernel_spmd(
        nc, in_maps, core_ids=list(range(N_CORES)),
        trace=trace,
        trace_cores=list(range(N_CORES)) if trace else None,
        stitch_traces=False,
    )
    LAST_RESULTS = res

    out = np.empty((T, H, D), dtype=np.float32)
    kc_full = np.empty((num_slots, NUM_KV_HEADS, D), dtype=np.float32)
    vc_full = np.empty((num_slots, NUM_KV_HEADS, D), dtype=np.float32)
    for c in range(N_CORES):
        r = res.results[c]
        out[:, c * HPC:(c + 1) * HPC, :] = r["out"].transpose(1, 0, 2)
        kc_full[:, c, :] = r["kc_out"]
        vc_full[:, c, :] = r["vc_out"]

    if not slots_arange:
        # general slot_mapping: redo the cache scatter on host (device path
        # assumed the arange fast path for the new-token region)
        kc_full = k_cache.copy()
        vc_full = v_cache.copy()
        valid = slots >= 0
        kc_full[slots[valid]] = k[valid]
        vc_full[slots[valid]] = v[valid]

    return out, kc_full, vc_full


# revision 32
# speedup vs baseline: 1.0705x; 1.0705x over previous
"""Trainium2 Bass kernel for variable-length prefill GQA attention + KV-cache store.

Problem (nn_Attention_44057774522397):
  q [T=4096, 32, 128] f32, k/v [T, 8, 128] f32, k_cache/v_cache [8192, 8, 128] f32,
  cu_seqlens [5] i32, slot_mapping [T] i32.
  reference: scatter k/v into caches at slot_mapping; causal varlen attention
  (GQA 32q/8kv heads) over the packed sequences.  Returns (out, k_cache, v_cache).

Sharding: 8 cores, head-parallel.  Core c gets q heads [4c:4c+4] and kv head c
(GQA group size 4 -> each core needs exactly one kv head).  slot/cu handled on
host (program structure); caches sharded by kv head.

Device kernel (per core, SPMD):
  - S^T = K_j^T-blocks x Q-stripes via f32r matmuls (N=512 full-speed, ~1.5e-4 rel),
    scores computed transposed [k_rows, q_cols] so no on-device transposes are
    needed anywhere (host pre-transposes q/k into [head_dim, tokens]).
  - P^T = exp(scale * S^T) on ScalarE, batched [128, 3*512] per op, output fp16.
  - causal masking: gpsimd affine_select zeroes invalid (q < k) entries of P^T.
  - PV: out[q,129] += P^T-chunk.T @ [V | ones] in fp16 (FWL weight loads);
    column 128 accumulates the softmax denominator for free.
  - normalize with DVE reciprocal + tensor_scalar_mul, DMA out.
  - cache updates are pure DMAs (slot_mapping is arange in practice).
"""

import math
import numpy as np

NUM_HEADS = 32
NUM_KV_HEADS = 8
HEAD_DIM = 128
SCALE = 1.0 / math.sqrt(128.0)
N_CORES = 8
HPC = NUM_HEADS // N_CORES          # q heads per core
GROUPS = NUM_HEADS // NUM_KV_HEADS  # GQA group size
BLK = 128
WBLKS = 4                           # stripe width in 128-blocks (512 cols)
JB = 2                              # j-blocks per exp batch (2 PSUM banks)

_PROGRAM_CACHE = {}
LAST_RESULTS = None  # BassKernelResults of the most recent device run


def _build_program(seq_lens, T, num_slots, skip_upper):
    import sys
    if "/opt/trn_rl_repo" not in sys.path:
        sys.path.insert(0, "/opt/trn_rl_repo")
    from contextlib import ExitStack
    import concourse.bass as bass
    import concourse.mybir as mybir
    import concourse.tile as tile

    f32 = mybir.dt.float32
    f32r = mybir.dt.float32r
    f16 = mybir.dt.float16

    nc = bass.Bass(trn_type="TRN2")
    qT = nc.dram_tensor("qT", [HPC, HEAD_DIM, T], f16, kind="ExternalInput")
    kT = nc.dram_tensor("kT", [HEAD_DIM, T], f16, kind="ExternalInput")
    vaug = nc.dram_tensor("vaug", [BLK, T // BLK, HEAD_DIM + 1], f16, kind="ExternalInput")
    knat = nc.dram_tensor("knat", [T, HEAD_DIM], f32, kind="ExternalInput")
    vnat = nc.dram_tensor("vnat", [T, HEAD_DIM], f32, kind="ExternalInput")
    kc_in = nc.dram_tensor("kc_in", [num_slots, HEAD_DIM], f32, kind="ExternalInput")
    vc_in = nc.dram_tensor("vc_in", [num_slots, HEAD_DIM], f32, kind="ExternalInput")
    out = nc.dram_tensor("out", [HPC, T, HEAD_DIM], f32, kind="ExternalOutput")
    kc_out = nc.dram_tensor("kc_out", [num_slots, HEAD_DIM], f32, kind="ExternalOutput")
    vc_out = nc.dram_tensor("vc_out", [num_slots, HEAD_DIM], f32, kind="ExternalOutput")

    # packed-sequence offsets (in 128-blocks); all lens are multiples of 128
    seqs = []
    o = 0
    for L in seq_lens:
        if L:
            seqs.append((o, L))
        o += L
    assert o == T

    with ExitStack() as ctx:
        tc = ctx.enter_context(tile.TileContext(nc))

        resident = ctx.enter_context(tc.tile_pool(name="resident", bufs=1))
        kT_r = resident.tile([BLK, T], f16)
        v_sb = resident.tile([BLK, T // BLK, HEAD_DIM + 1], f16)

        # warm the exp table before anything else
        warm = resident.tile([BLK, 1], f32)
        nc.vector.memset(warm, 0.0)
        nc.scalar.activation(out=warm, in_=warm,
                             func=mybir.ActivationFunctionType.Exp, scale=1.0)

        # chunked input loads, in first-use order; q is streamed per stripe
        # (prefetched one stripe ahead inside the main loop)
        IC = T // 4
        nb_all = T // BLK
        prologue_loads = []
        for lo in range(0, T, IC):
            prologue_loads.append((kT_r[:, lo:lo + IC], kT[:, lo:lo + IC]))
            jlo, jhi = lo // BLK, (lo + IC) // BLK
            prologue_loads.append((v_sb[:, jlo:jhi, :], vaug[:, jlo:jhi, :]))
        nc.sync.dma_start(out=prologue_loads[0][0], in_=prologue_loads[0][1])

        # ---- cache copy-through (pure DMA; spread through the main loop
        # so it fills DMA idle slots instead of bursting against the
        # attention input loads on chip-shared HBM) ----
        CH = 512
        cache_dmas = []
        for lo in range(0, T, CH):
            cache_dmas.append((kc_out[lo:lo + CH, :], knat[lo:lo + CH, :]))
            cache_dmas.append((vc_out[lo:lo + CH, :], vnat[lo:lo + CH, :]))
        if not skip_upper:
            for lo in range(T, num_slots, CH):
                cache_dmas.append((kc_out[lo:lo + CH, :], kc_in[lo:lo + CH, :]))
                cache_dmas.append((vc_out[lo:lo + CH, :], vc_in[lo:lo + CH, :]))
        cache_dmas.reverse()  # pop() from the front order

        ppool = ctx.enter_context(tc.tile_pool(name="ppool", bufs=4))
        qpool = ctx.enter_context(tc.tile_pool(name="qpool", bufs=4))
        opool = ctx.enter_context(tc.tile_pool(name="opool", bufs=4))
        spsum = ctx.enter_context(tc.tile_pool(name="spsum", bufs=2, space="PSUM"))
        vpsum = ctx.enter_context(tc.tile_pool(name="vpsum", bufs=4, space="PSUM"))

        # stripe list: (head, seq offset tokens, first i-block, last i-block+1)
        stripes = []
        for h in range(HPC):
            for (o, L) in seqs:
                nb = L // BLK
                for ib0 in range(0, nb, WBLKS):
                    stripes.append((h, o, ib0, min(ib0 + WBLKS, nb)))

        # max P^T tile free size: nj * w is maximized by full stripes: 8 * 512
        MAXPT = 8 * (WBLKS * BLK)

        def emit_batch(stripe, pt, qs, jb):
            """One S^T batch: <=JB k-block matmuls + one exp + diag masks."""
            h, o, ib0, ib1 = stripe
            w = (ib1 - ib0) * BLK
            nj = ib1
            ptv = pt.rearrange("p a b -> p (a b)")
            bsz = min(JB, nj - jb)
            # columns < d_min are never read by any PV group (those (i, j)
            # pairs are anti-causal), so neither compute nor exp them
            d_min = max(0, jb - ib0) * BLK
            stp = spsum.tile([BLK, JB, WBLKS * BLK], f32, tag="stp")
            for jj in range(bsz):
                j = jb + jj
                k0 = o + j * BLK
                nc.tensor.matmul(
                    stp[:, jj, d_min:w],
                    kT_r[:, k0:k0 + BLK],
                    qs[:, d_min:w],
                    start=True, stop=True,
                )
            pt_slice = ptv[:, jb * w:(jb + bsz) * w].rearrange(
                "p (a b) -> p a b", a=bsz)[:, :, d_min:w]
            nc.scalar.activation(
                out=pt_slice,
                in_=stp[:, 0:bsz, d_min:w],
                func=mybir.ActivationFunctionType.Exp,
                scale=float(SCALE),
            )
            # causal mask for any diagonal j in this batch: only the
            # [d, d+128) window is ever read partially masked
            for j in range(max(jb, ib0), jb + bsz):
                d = (j - ib0) * BLK
                pj = ptv[:, j * w + d:j * w + d + BLK]
                nc.gpsimd.affine_select(
                    out=pj, in_=pj,
                    compare_op=mybir.AluOpType.is_ge,
                    fill=0.0, base=0, channel_multiplier=-1,
                    pattern=[[1, BLK]],
                )

        def emit_out_dma(stripe, ob):
            h, o, ib0, ib1 = stripe
            wb = ib1 - ib0
            t0 = o + ib0 * BLK
            nc.sync.dma_start(
                out=out[h, t0:t0 + wb * BLK, :].rearrange(
                    "(a p) d -> p a d", p=BLK),
                in_=ob[:, 0:wb, :],
            )

        # --- matmul-granular software pipeline ------------------------------
        # The exp stream on ScalarE is the bottleneck; the PE FIFO must
        # deliver S^T batches at exp cadence while filling the gaps with PV
        # matmuls.  Pace PV emission at single-matmul granularity so that
        # seq-tail stripes (large triangular PV load) don't burst-starve the
        # exp stream.  Stripe distance <= 3 (pt pool has 4 bufs).
        n_batches_total = sum((s[3] + JB - 1) // JB for s in stripes)
        n_pv_mms_total = sum((i + 1) for s in stripes for i in range(s[2], s[3]))
        ratio = n_pv_mms_total / max(n_batches_total, 1)

        from collections import deque
        bq = deque()          # stripes with A fully emitted, B pending
        cur_b = None          # [stripe, pt, ob, pv, i, j]
        done_b_stripes = [0]
        emitted_a = [0]
        emitted_pv = [0]

        def emit_next_pv_mm():
            nonlocal cur_b
            if cur_b is None:
                if not bq:
                    return False
                stripe, pt = bq.popleft()
                ob = opool.tile([BLK, WBLKS, HEAD_DIM], f32, tag="ob")
                cur_b = [stripe, pt, ob, None, stripe[2], 0]
            stripe, pt, ob, pv, i, j = cur_b
            h, o, ib0, ib1 = stripe
            w = (ib1 - ib0) * BLK
            ptv = pt.rearrange("p a b -> p (a b)")
            ic = (i - ib0) * BLK
            if pv is None:
                pv = vpsum.tile([BLK, HEAD_DIM + 1], f32, tag="pv")
                cur_b[3] = pv
            nc.tensor.matmul(
                pv,
                ptv[:, j * w + ic:j * w + ic + BLK],
                v_sb[:, o // BLK + j, :],
                start=(j == 0), stop=(j == i),
            )
            emitted_pv[0] += 1
            if j < i:
                cur_b[5] = j + 1
                return True
            # group done: normalize into ob
            recip = opool.tile([BLK, 1], f32, tag="recip")
            nc.vector.reciprocal(recip, pv[:, HEAD_DIM:HEAD_DIM + 1])
            nc.vector.tensor_scalar_mul(
                ob[:, i - ib0, :], pv[:, 0:HEAD_DIM], recip)
            if i + 1 < ib1:
                cur_b[3] = None
                cur_b[4] = i + 1
                cur_b[5] = 0
            else:
                emit_out_dma(stripe, ob)
                cur_b = None
                done_b_stripes[0] += 1
            return True

        n_stripes = len(stripes)
        cd_per_stripe = (len(cache_dmas) + n_stripes - 1) // max(n_stripes - 4, 1)

        qs_tiles = {}

        def prefetch_qs(si):
            if si >= n_stripes or si in qs_tiles:
                return
            h, o, ib0, ib1 = stripes[si]
            w = (ib1 - ib0) * BLK
            q0 = o + ib0 * BLK
            qs = qpool.tile([BLK, WBLKS * BLK], f16, tag="qs")
            nc.sync.dma_start(out=qs[:, 0:w], in_=qT[h, :, q0:q0 + w])
            qs_tiles[si] = qs

        prefetch_qs(0)
        # rest of the prologue loads, after the first q stripe
        for dst, srcap in prologue_loads[1:]:
            nc.sync.dma_start(out=dst, in_=srcap)
        prefetch_qs(1)

        for si, stripe in enumerate(stripes):
            h, o, ib0, ib1 = stripe
            pt = ppool.tile([BLK, MAXPT // BLK, BLK], f16, tag="pt")
            prefetch_qs(si + 1)
            prefetch_qs(si + 2)
            qs = qs_tiles.pop(si)
            if si >= 2:
                for _ in range(cd_per_stripe):
                    if cache_dmas:
                        dst, srcap = cache_dmas.pop()
                        nc.sync.dma_start(out=dst, in_=srcap)
            for jb in range(0, ib1, JB):
                emit_batch(stripe, pt, qs, jb)
                emitted_a[0] += 1
                while (emitted_pv[0] + 1) <= ratio * (emitted_a[0] - 1):
                    if not emit_next_pv_mm():
                        break
            bq.append((stripe, pt))
            # respect the pt pool depth: at most 3 stripes between the one
            # being A-emitted and the oldest not-fully-consumed one
            while si + 1 - done_b_stripes[0] >= 4:
                if not emit_next_pv_mm():
                    break
        while emit_next_pv_mm():
            pass
        while cache_dmas:
            dst, srcap = cache_dmas.pop()
            nc.sync.dma_start(out=dst, in_=srcap)

    return nc


def _strip_redundant_pe_self_waits(nc, mybir):
    """Remove PE-engine waits on the PE's own completion semaphore.

    Tile emits them when a PSUM slot is rewritten (WAW vs the slot's previous
    tile), but every such slot also carries a wait on the previous tile's
    last *reader* (ACT exp / DVE normalize), and that reader itself waited on
    the old writers -- so the own-sem wait is transitively implied.  Keeping
    it forces the PE sequencer to drain its 64-deep pipeline at every slot
    reuse, serializing the matmul stream."""
    pe = mybir.EngineType.PE
    n = 0
    for func in nc.m.functions:
        for block in func.blocks:
            for inst in block.instructions:
                if inst.engine != pe:
                    continue
                si = inst.sync_info
                if si is None or not si.on_wait:
                    continue
                keep = [w for w in si.on_wait
                        if not (getattr(w, "ant_name", "") or "").startswith("PE_")]
                if len(keep) != len(si.on_wait):
                    inst.sync_info = mybir.SyncInfo(
                        on_wait=keep, on_update=list(si.on_update))
                    n += 1
    return n


def _trim_exit_barrier(nc):
    """Drop the duplicated trailing all-engine barrier after the semaphore
    clear (bass emits the drain+barrier+clear+barrier sequence twice "to be
    safe"); the second EVSEM butterfly costs ~4-6us of pure epilogue."""
    for func in nc.m.functions:
        for block in func.blocks:
            if not block.name.endswith("_end"):
                continue
            idx = None
            for n, inst in enumerate(block.instructions):
                if "EVENT_SEMAPHORE_RANGE_CLEAR" in type(inst).__name__ or                    "RANGE_CLEAR" in str(getattr(inst, "isa_opcode_name", "")) or                    "RANGE_CLEAR" in inst.concise():
                    idx = n
            if idx is not None and idx + 1 < len(block.instructions):
                block.instructions = block.instructions[:idx + 1]
    return


def _legalize_waits(nc, mybir, max_waits=1):
    n_fixed = 0
    for func in nc.m.functions:
        for block in func.blocks:
            new_list = []
            changed = False
            for inst in block.instructions:
                si = inst.sync_info
                if si is not None and si.on_wait and len(si.on_wait) > max_waits:
                    waits = list(si.on_wait)
                    head, tail = waits[:-max_waits], waits[-max_waits:]
                    while head:
                        chunk, head = head[:max_waits], head[max_waits:]
                        nop = mybir.InstNoOp(
                            name=f"waitsplit-{nc.next_id()}", ins=[], outs=[])
                        nop.engine = inst.engine
                        nop.sync_info = mybir.SyncInfo(on_wait=chunk, on_update=[])
                        new_list.append(nop)
                        nc.register_instruction(nop, overwrite=True)
                    inst.sync_info = mybir.SyncInfo(
                        on_wait=tail, on_update=list(si.on_update))
                    changed = True
                    n_fixed += 1
                new_list.append(inst)
            if changed:
                block.instructions = new_list
    return n_fixed


def _get_program(seq_lens, T, num_slots, skip_upper):
    import sys
    if "/opt/trn_rl_repo" not in sys.path:
        sys.path.insert(0, "/opt/trn_rl_repo")
    import concourse.mybir as mybir

    key = (tuple(seq_lens), T, num_slots, skip_upper)
    if key not in _PROGRAM_CACHE:
        nc = _build_program(seq_lens, T, num_slots, skip_upper)
        _strip_redundant_pe_self_waits(nc, mybir)
        _trim_exit_barrier(nc)
        _legalize_waits(nc, mybir, max_waits=1)
        _PROGRAM_CACHE[key] = nc
    return _PROGRAM_CACHE[key]


def _host_reference(q, k, v, k_cache, v_cache, cu_seqlens, slot_mapping):
    """Pure-numpy fallback for input shapes the device program doesn't cover."""
    T = q.shape[0]
    kc = k_cache.copy()
    vc = v_cache.copy()
    valid = slot_mapping >= 0
    kc[slot_mapping[valid]] = k[valid]
    vc[slot_mapping[valid]] = v[valid]
    groups = q.shape[1] // k.shape[1]
    kk = np.repeat(k, groups, axis=1)
    vv = np.repeat(v, groups, axis=1)
    out = np.zeros_like(q)
    for b in range(len(cu_seqlens) - 1):
        s0, s1 = int(cu_seqlens[b]), int(cu_seqlens[b + 1])
        L = s1 - s0
        if L <= 0:
            continue
        qs = q[s0:s1].astype(np.float64)
        ks = kk[s0:s1].astype(np.float64)
        vs = vv[s0:s1].astype(np.float64)
        sc = np.einsum("qhd,khd->hqk", qs, ks) * SCALE
        mask = np.tril(np.ones((L, L), dtype=bool))
        sc = np.where(mask[None], sc, -np.inf)
        sc -= sc.max(axis=-1, keepdims=True)
        p = np.exp(sc)
        p /= p.sum(axis=-1, keepdims=True)
        out[s0:s1] = np.einsum("hqk,khd->qhd", p, vs).astype(q.dtype)
    return out, kc, vc


def _install_ntff_hook():
    """Make `antenv.axon_hooks` importable so run_bass_kernel_spmd(trace=True)
    can capture NTFF profiles under axon.  The image's antenv stub lacks the
    module; recreate it and register the ctypes-based hook from trn_boot.
    Silently degrades (tracing skipped) on any failure."""
    import sys
    import types
    try:
        import antenv  # noqa: F401
        if "antenv.axon_hooks" in sys.modules:
            return
        mod = types.ModuleType("antenv.axon_hooks")
        mod._hook = None

        def set_axon_ntff_profile_hook(h):
            mod._hook = h

        def get_axon_ntff_profile_hook():
            return mod._hook

        mod.set_axon_ntff_profile_hook = set_axon_ntff_profile_hook
        mod.get_axon_ntff_profile_hook = get_axon_ntff_profile_hook
        sys.modules["antenv.axon_hooks"] = mod
        try:
            if "/root/.axon_site" not in sys.path:
                sys.path.append("/root/.axon_site")
            from trn_agent_boot.trn_boot import _ntff_profile_via_ctypes
            mod._hook = _ntff_profile_via_ctypes("/opt/axon/libaxon_pjrt.so")
        except Exception:
            pass
    except Exception:
        pass


def kernel(q, k, v, k_cache, v_cache, cu_seqlens, slot_mapping):
    global LAST_RESULTS
    import os
    import sys
    if "/opt/trn_rl_repo" not in sys.path:
        sys.path.insert(0, "/opt/trn_rl_repo")
    _install_ntff_hook()

    q = np.asarray(q, dtype=np.float32)
    k = np.asarray(k, dtype=np.float32)
    v = np.asarray(v, dtype=np.float32)
    k_cache = np.asarray(k_cache, dtype=np.float32)
    v_cache = np.asarray(v_cache, dtype=np.float32)
    cu = np.asarray(cu_seqlens, dtype=np.int64)
    slots = np.asarray(slot_mapping, dtype=np.int64)

    T, H, D = q.shape
    num_slots = k_cache.shape[0]
    seq_lens = (cu[1:] - cu[:-1]).tolist()

    device_ok = (
        H == NUM_HEADS and D == HEAD_DIM and k.shape[1] == NUM_KV_HEADS
        and T % BLK == 0 and all(L % BLK == 0 and L >= 0 for L in seq_lens)
        and int(cu[0]) == 0 and int(cu[-1]) == T
    )
    slots_arange = bool(np.array_equal(slots, np.arange(T)))
    if not device_ok:
        return _host_reference(q, k, v, k_cache, v_cache, cu_seqlens, slot_mapping)

    from concourse.bass_utils import run_bass_kernel_spmd

    skip_upper = (not np.any(k_cache[T:])) and (not np.any(v_cache[T:]))
    nc = _get_program(tuple(int(L) for L in seq_lens), T, num_slots,
                      bool(skip_upper))

    in_maps = []
    for c in range(N_CORES):
        hs = slice(c * HPC, (c + 1) * HPC)
        qc = q[:, hs, :]                             # [T, 4, 128]
        qT = np.ascontiguousarray(qc.transpose(1, 2, 0)).astype(np.float16)
        kc = k[:, c, :]                              # [T, 128]
        kT = np.ascontiguousarray(kc.T).astype(np.float16)
        vc = v[:, c, :]                              # [T, 128]
        vaug = np.ones((BLK, T // BLK, HEAD_DIM + 1), dtype=np.float16)
        vaug[:, :, :HEAD_DIM] = vc.reshape(T // BLK, BLK, HEAD_DIM).transpose(
            1, 0, 2).astype(np.float16)
        in_maps.append({
            "qT": qT,
            "kT": kT,
            "vaug": vaug,
            "knat": np.ascontiguousarray(kc),
            "vnat": np.ascontiguousarray(vc),
            "kc_in": np.ascontiguousarray(k_cache[:, c, :]),
            "vc_in": np.ascontiguousarray(v_cache[:, c, :]),
        })

    trace = bool(int(os.environ.get("KERNEL_TRACE", "0")))
    res = run_bass_kernel_spmd(
        nc, in_maps, core_ids=list(range(N_CORES)),
        trace=trace,
        trace_cores=list(range(N_CORES)) if trace else None,
        stitch_traces=False,
    )
    LAST_RESULTS = res

    out = np.empty((T, H, D), dtype=np.float32)
    kc_full = np.empty((num_slots, NUM_KV_HEADS, D), dtype=np.float32)
    vc_full = np.empty((num_slots, NUM_KV_HEADS, D), dtype=np.float32)
    for c in range(N_CORES):
        r = res.results[c]
        out[:, c * HPC:(c + 1) * HPC, :] = r["out"].transpose(1, 0, 2)
        kc_full[:, c, :] = r["kc_out"]
        vc_full[:, c, :] = r["vc_out"]

    if not slots_arange:
        # general slot_mapping: redo the cache scatter on host (device path
        # assumed the arange fast path for the new-token region)
        kc_full = k_cache.copy()
        vc_full = v_cache.copy()
        valid = slots >= 0
        kc_full[slots[valid]] = k[valid]
        vc_full[slots[valid]] = v[valid]

    return out, kc_full, vc_full
